# revision 1
# baseline (speedup 1.0000x reference)
"""Trainium2 Bass kernel for ragged multi-head self-attention (8 NeuronCores).

Reference: per ragged segment (offsets delimit segments of x):
    q,k,v = x@Wq, x@Wk, x@Wv (per-token), softmax(q k^T / sqrt(dh)) v within
    the segment per head, then out = attn@Wo + bo.

Distribution: work is sharded by (segment, head-pair).  Heads are
independent in attention, and the projections split cleanly by head
(column blocks of Wq/Wk/Wv, row blocks of Wo), so a core that owns 2 of
the 8 heads of a segment does 1/4 of that segment's projection and
attention work and emits a PARTIAL output (its heads' contribution to
attn@Wo); the host sums partials and adds bo.  All 8 cores run the same
graph (a "template" of positions, each position = one (C kv-tiles,
S q-tiles) capacity); the host packs each core's (segment, head-pair)
instances into the template and supplies per-core head-PERMUTED weights
so position p always reads weight columns [p*128,(p+1)*128).

Raggedness is handled with exact zero-cost masking: padded key tokens
are zeros (=> scores 0 => exp=1) and a per-token validity column
appended to V zeroes their contribution to numerator and denominator;
the validity column produces the softmax denominator via the same PE
matmul that computes attn^T = V_aug^T @ exp(S^T).  No running max is
needed (scores are O(9), exp cannot overflow in fp32).

All matmuls bf16 (host pre-casts inputs) with fp32 PSUM accumulation.
exp runs on the scalar engine straight out of PSUM.
"""

import os
from contextlib import ExitStack

import numpy as np

D = 512
H = 8
DH = 64
NH = 2        # heads per position (head-pair)
P = 128
QT = 128      # query tile
CK = 128      # kv chunk
N_CORES = 8
EXPW = int(os.environ.get("K_EXPW", "1024"))  # exp batch width
SC_BUFS = int(os.environ.get("K_SC_BUFS", "2"))   # score buffers
PIPE_DEPTH = int(os.environ.get("K_PIPE", "4"))  # score batches ahead of AV


# --------------------------------------------------------------------------
# schedule construction (host, from runtime offsets)
# --------------------------------------------------------------------------

def build_schedule(offsets, n_cores=N_CORES):
    offsets = np.asarray(offsets).astype(np.int64)
    lengths = np.diff(offsets)
    assert (lengths > 0).all(), "zero-length segments unsupported"
    segs = [(i, int(offsets[i]), int(L), (int(L) + CK - 1) // CK)
            for i, L in enumerate(lengths)]

    # instances: (si, st, L, C, hp) -- whole segment, one head-pair
    insts = [(si, st, L, C, hp) for (si, st, L, C) in segs
             for hp in range(H // NH)]
    # order: C desc; within an equal-C run, hp-pair-major so that
    # consecutive positions get the SAME segment per core slot (enables
    # kv aliasing: position p+1 reuses p's x block).
    insts.sort(key=lambda t: (-t[3], t[4] // 2, t[0], t[4] % 2))

    positions = [insts[i:i + n_cores] for i in range(0, len(insts), n_cores)]
    template = []
    for p, pos in enumerate(positions):
        Cp = max(t[3] for t in pos)
        alias = (p > 0 and len(pos) == len(positions[p - 1])
                 and all(a[0] == b[0] and a[3] == b[3]
                         for a, b in zip(pos, positions[p - 1]))
                 and template[p - 1][0] == Cp)
        template.append((Cp, Cp, bool(alias)))

    core_groups = [[None] * len(template) for _ in range(n_cores)]
    for p, pos in enumerate(positions):
        for c, inst in enumerate(pos):
            core_groups[c][p] = inst

    NKV = sum(C * CK for (C, S, alias) in template if not alias)
    NQP = sum(S * QT for (C, S, alias) in template)
    return dict(template=tuple(template), core_groups=core_groups,
                NKV=NKV, NQP=NQP, segs=segs)


def kv_offsets(template):
    # kv block start (cols) per position, honoring aliasing
    offs = []
    o = 0
    for (C, S, alias) in template:
        if alias:
            offs.append(offs[-1])
        else:
            offs.append(o)
            o += C * CK
    return offs


def shard_inputs(x, Wq, Wk, Wv, Wo, sched, n_cores=N_CORES):
    import ml_dtypes
    BF = ml_dtypes.bfloat16
    T, D_ = x.shape
    template = sched["template"]
    NKV = sched["NKV"]
    kvoffs = kv_offsets(template)
    xkvT = np.zeros((n_cores, D_, NKV), BF)
    validity = np.zeros((n_cores, NKV), np.float32)
    Wqp = np.zeros((n_cores, D_, len(template) * NH * DH), BF)
    Wkp = np.zeros_like(Wqp)
    Wvp = np.zeros_like(Wqp)
    Wop = np.zeros((n_cores, len(template) * NH * DH, D_), BF)
    scatter = []  # (core, prow0, grow0, nrows) partial-out accumulation
    xT = np.ascontiguousarray(x.T)
    for c in range(n_cores):
        q0 = 0
        for p, ((C, S, alias), inst) in enumerate(
                zip(template, sched["core_groups"][c])):
            wcol = p * NH * DH
            if inst is not None:
                si, st, L, Ci, hp = inst
                if not alias:
                    kv0 = kvoffs[p]
                    xkvT[c, :, kv0:kv0 + L] = xT[:, st:st + L]
                    validity[c, kv0:kv0 + L] = 1.0
                hcol = hp * NH * DH
                Wqp[c, :, wcol:wcol + NH * DH] = Wq[:, hcol:hcol + NH * DH]
                Wkp[c, :, wcol:wcol + NH * DH] = Wk[:, hcol:hcol + NH * DH]
                Wvp[c, :, wcol:wcol + NH * DH] = Wv[:, hcol:hcol + NH * DH]
                Wop[c, wcol:wcol + NH * DH, :] = Wo[hcol:hcol + NH * DH, :]
                scatter.append((c, q0, st, min(L, S * QT)))
            elif not alias:
                validity[c, kvoffs[p]] = 1.0  # keep denominators > 0
            q0 += S * QT
    return xkvT, validity, Wqp, Wkp, Wvp, Wop, scatter


# --------------------------------------------------------------------------
# device graph
# --------------------------------------------------------------------------

def build_graph(template, NKV, NQP, repeat=1):
    # repeat>1 emits the whole body N times inside one NEFF (used only by
    # the timing probe; shared tile tags serialize repeats via hazards).
    import concourse.bass as bass
    import concourse.tile as tile
    from concourse import bacc, library_config, mybir

    F32 = mybir.dt.float32
    BF16 = mybir.dt.bfloat16
    EXP = mybir.ActivationFunctionType.Exp

    NPOS = len(template)
    NKVT = NKV // CK
    WCOLS = NPOS * NH * DH
    CMAX = max(C for C, S, a in template)
    SMAX = max(S for C, S, a in template)
    kvoffs = kv_offsets(template)
    nc = bacc.Bacc("TRN2", target_bir_lowering=False, debug=False)

    xkvT_d = nc.dram_tensor("xkvT", [D, NKV], BF16, kind="ExternalInput")
    val_d = nc.dram_tensor("validity", [NKV], F32, kind="ExternalInput")
    W_d = {w: nc.dram_tensor(w, [D, WCOLS], BF16, kind="ExternalInput")
           for w in ("Wq", "Wk", "Wv")}
    W_d["Wo"] = nc.dram_tensor("Wo", [WCOLS, D], BF16, kind="ExternalInput")
    out_d = nc.dram_tensor("out", [NQP, D], BF16, kind="ExternalOutput")

    with ExitStack() as ctx:
        tc = ctx.enter_context(tile.TileContext(nc))
        nc.gpsimd.load_library(library_config.attnmlp)

        # ---- persistent SBUF tensors ----
        singles = ctx.enter_context(tc.tile_pool(name="singles", bufs=1))

        def single(shape, dtype, name):
            return singles.tile(shape, dtype, name=name, tag=name)

        W_bf = {w: single([P, 4, WCOLS], BF16, f"{w}_bf")
                for w in ("Wq", "Wk", "Wv")}
        Wo_bf = single([P, NPOS, D], BF16, "Wo_bf")
        xkvT_bf = single([P, 4, NKV], BF16, "xkvT_bf")
        val_sb = single([P, NKVT], F32, "val_sb")

        # ---- per-position double-buffered tensors ----
        pkt = ctx.enter_context(tc.tile_pool(name="pkt", bufs=2))
        pqt = ctx.enter_context(tc.tile_pool(name="pqt", bufs=2))
        pv = ctx.enter_context(tc.tile_pool(name="pv", bufs=2))
        pattn = ctx.enter_context(tc.tile_pool(name="pattn", bufs=2))

        # ---- pools ----
        psc = ctx.enter_context(
            tc.tile_pool(name="psc", bufs=SC_BUFS, space="PSUM"))
        pat = ctx.enter_context(
            tc.tile_pool(name="pat", bufs=2, space="PSUM"))
        ppt = ctx.enter_context(tc.tile_pool(name="ppt", bufs=6))
        prep = ctx.enter_context(tc.tile_pool(name="prep", bufs=4))
        pout = ctx.enter_context(tc.tile_pool(name="pout", bufs=6))

        for _rep in range(repeat):
            # DMA order = need order: Wk+Wq unlock the first scores, then
            # position-0 x, then Wv (first AV), Wo, remaining x.
            def load_w(w):
                nc.sync.dma_start(
                    out=W_bf[w],
                    in_=W_d[w][:, :].rearrange("(dc p) c -> p dc c", p=P))

            load_w("Wk")
            nc.sync.dma_start(
                out=val_sb, in_=val_d[:].rearrange("(t p) -> p t", p=P))

            XSTG = 2048

            def load_range(lo, hi):
                o = lo
                first = True
                while o < hi:
                    n = min(512 if first else XSTG, hi - o)
                    first = False
                    for dc in range(4):
                        nc.sync.dma_start(
                            out=xkvT_bf[:, dc, o:o + n],
                            in_=xkvT_d[dc * P:(dc + 1) * P, o:o + n])
                    o += n

            # first x chunk before Wq's DMA: k-proj block 0 overlaps it
            first_n = min(512, template[0][0] * CK)
            load_range(kvoffs[0], kvoffs[0] + first_n)
            load_w("Wq")
            for p, (C, S, alias) in enumerate(template):
                if not alias:
                    lo = kvoffs[p] + (first_n if p == 0 else 0)
                    load_range(lo, kvoffs[p] + C * CK)
                if p == 0:
                    load_w("Wv")
                    nc.sync.dma_start(
                        out=Wo_bf,
                        in_=W_d["Wo"][:, :].rearrange(
                            "(pos p) d -> p pos d", p=P))

            # ---- need-driven keyed emission ----
            pending = []      # key emission order
            pending_map = {}  # key -> emit closure

            def put(key, fn):
                pending.append(key)
                pending_map[key] = fn

            def emit_key(key):
                fn = pending_map.pop(key, None)
                if fn is not None:
                    fn()

            def drain_one():
                while pending:
                    key = pending.pop(0)
                    if key in pending_map:
                        emit_key(key)
                        return

            def drain_all():
                while pending:
                    key = pending.pop(0)
                    if key in pending_map:
                        emit_key(key)

            def need(keys):
                for k in keys:
                    if k in pending_map:
                        emit_key(k)

            def col_blocks(lo, hi, w=512):
                o = lo
                while o < hi:
                    yield o, min(w, hi - o)
                    o += w

            # per-position tiles, created at registration time
            pos_tiles = {}

            def kq_keys(p, c0t, nbt, qo, qn):
                C = template[p][0]
                keys = []
                lo, hi = c0t * CK, (c0t + nbt) * CK
                for b0, n in col_blocks(0, C * CK):
                    if b0 < hi and b0 + n > lo:
                        keys.append(("k", p, b0))
                for b0, n in col_blocks(0, template[p][1] * QT):
                    if b0 < qo + qn and b0 + n > qo:
                        keys.append(("q", p, b0))
                return keys

            def v_keys(p, c0t, nbt):
                return ([("v", p, i) for i in range(c0t, c0t + nbt)]
                        + [("val", p)])

            def proj_items(p):
                C, S, alias = template[p]
                kv_lo = kvoffs[p]
                wc = p * P  # weight column block (NH*DH == P)
                kT = pkt.tile([P, CMAX * CK], BF16, tag="kT",
                              name=f"kT_{p}")
                qTt = pqt.tile([P, SMAX * QT], BF16, tag="qT",
                               name=f"qT_{p}")
                vt = pv.tile([P, CMAX, NH, 66], BF16, tag="v",
                             name=f"v_{p}")
                pos_tiles[p] = (kT, qTt, vt)

                def kq_emit(dst, w, b0, n):
                    ps = psc.tile([P, 512], F32, tag="pj", bufs=2,
                                  name="kq_ps")
                    for dc in range(4):
                        nc.tensor.matmul(
                            ps[:, :n],
                            lhsT=W_bf[w][:, dc, wc:wc + P],
                            rhs=xkvT_bf[:, dc, kv_lo + b0:kv_lo + b0 + n],
                            start=(dc == 0), stop=(dc == 3))
                    nc.vector.tensor_copy(out=dst[:, b0:b0 + n],
                                          in_=ps[:, :n])

                for b0, n in col_blocks(0, C * CK):
                    put(("k", p, b0),
                        lambda b0=b0, n=n: kq_emit(kT, "Wk", b0, n))
                for b0, n in col_blocks(0, S * QT):
                    put(("q", p, b0),
                        lambda b0=b0, n=n: kq_emit(qTt, "Wq", b0, n))

                def emit_val():
                    t0 = kv_lo // CK
                    vap = val_sb[:, t0:t0 + C]
                    rep2 = bass.AP(tensor=vap.tensor, offset=vap.offset,
                                   ap=list(vap.ap[:2]) + [[0, NH]])
                    nc.vector.tensor_copy(out=vt[:, 0:C, :, DH], in_=rep2)
                put(("val", p), emit_val)

                for t in range(C):
                    def emit(t=t):
                        ps = psc.tile([P, 512], F32, tag="pj", bufs=2,
                                      name="v_ps")
                        for dc in range(4):
                            nc.tensor.matmul(
                                ps[:, :P],
                                lhsT=xkvT_bf[:, dc,
                                             kv_lo + t * CK:
                                             kv_lo + (t + 1) * CK],
                                rhs=W_bf["Wv"][:, dc, wc:wc + P],
                                start=(dc == 0), stop=(dc == 3))
                        nc.vector.tensor_copy(
                            out=vt[:, t, :, 0:DH],
                            in_=ps[:, :P].rearrange("p (h d) -> p h d", h=NH))
                    put(("v", p, t), emit)

            # ---- attention per position ----
            q0 = 0
            proj_items(0)
            for p, (C, S, alias) in enumerate(template):
                nq = S * QT
                if p + 1 < NPOS:
                    proj_items(p + 1)
                kT, qTt, vt = pos_tiles.pop(p)
                qbs = []
                o = 0
                for wdt in (512, 256, 128):
                    while nq - o >= wdt:
                        qbs.append((o, wdt))
                        o += wdt
                nqp = len(qbs) * 512
                attn_bf = pattn.tile([P, nqp], BF16, tag="attn_bf",
                                     name=f"attn_bf_{p}")
                batches = []  # (hl, qslot, qo, qn, c0, nb)
                for qslot, (qo, qn) in enumerate(qbs):
                    for hl in range(NH):
                        per_batch = EXPW // qn
                        c = 0
                        while c < C:
                            nb = min(per_batch, C - c)
                            batches.append((hl, qslot, qo, qn, c, nb))
                            c += nb
                at_tiles = {}

                def oproj_item(qt, p=p, qbs=qbs, attn_bf=attn_bf, q0=q0):
                    def emit():
                        j = next(j for j, (qo, qn) in enumerate(qbs)
                                 if qo <= qt * QT < qo + qn)
                        pcol = j * 512 + (qt * QT - qbs[j][0])
                        po = psc.tile([P, D], F32, tag="pj", bufs=2,
                                      name=f"po_{p}_{qt}")
                        nc.tensor.matmul(
                            po,
                            lhsT=attn_bf[:, pcol:pcol + QT],
                            rhs=Wo_bf[:, p, :],
                            start=True, stop=True)
                        osb = pout.tile([P, D], BF16, tag="osb",
                                        name=f"osb_{p}_{qt}")
                        nc.vector.tensor_copy(out=osb[:], in_=po)
                        nc.sync.dma_start(
                            out=out_d[q0 + qt * QT:q0 + (qt + 1) * QT, :],
                            in_=osb)
                    return emit

                def emit_scores(b, kT=kT, qTt=qTt, p=p):
                    hl, qslot, qo, qn, c0, nb = b
                    sc = psc.tile([P, EXPW], F32, tag="sc", name=f"sc_{p}")
                    pt = ppt.tile([P, EXPW], BF16, tag="pt", name=f"pt_{p}")
                    for i in range(nb):
                        nc.tensor.matmul(
                            sc[:, i * qn:(i + 1) * qn],
                            lhsT=kT[hl * DH:(hl + 1) * DH,
                                    (c0 + i) * CK:(c0 + i + 1) * CK],
                            rhs=qTt[hl * DH:(hl + 1) * DH, qo:qo + qn],
                            start=True, stop=True)
                    w = nb * qn
                    nc.scalar.activation(out=pt[:, :w], in_=sc[:, :w],
                                         func=EXP, scale=DH ** -0.5)
                    return pt

                def emit_av(b, pt, vt=vt, attn_bf=attn_bf, p=p, C=C,
                            S=S, qbs=qbs):
                    hl, qslot, qo, qn, c0, nb = b
                    if (hl, qslot) not in at_tiles:
                        at_tiles[(hl, qslot)] = pat.tile(
                            [DH + 1, 512], F32, tag="at", name=f"at_{p}_{hl}")
                    at = at_tiles[(hl, qslot)]
                    for i in range(nb):
                        nc.tensor.matmul(
                            at[:, 0:qn],
                            lhsT=vt[:, c0 + i, hl, 0:DH + 1],
                            rhs=pt[:, i * qn:(i + 1) * qn],
                            start=(c0 + i == 0), stop=(c0 + i == C - 1),
                            skip_group_check=True)
                    if c0 + nb == C:
                        # normalize straight out of the PSUM accumulator
                        del at_tiles[(hl, qslot)]
                        recip = prep.tile([1, 512], F32, tag="recip",
                                          name=f"recip_{p}_{hl}")
                        rep = prep.tile([DH, 512], F32, tag="rep",
                                        name=f"rep_{p}_{hl}")
                        nc.vector.reciprocal(
                            out=recip[:, :qn], in_=at[DH:DH + 1, 0:qn])
                        nc.gpsimd.partition_broadcast(
                            rep[:, :qn], recip[:, :qn], channels=DH)
                        nc.vector.tensor_mul(
                            out=attn_bf[hl * DH:(hl + 1) * DH,
                                        qslot * 512:qslot * 512 + qn],
                            in0=at[0:DH, 0:qn], in1=rep[:, :qn])
                        if hl == NH - 1:
                            qo0, qn0 = qbs[qslot]
                            qts = [qt for qt in range(S)
                                   if qo0 <= qt * QT < qo0 + qn0]
                            last = (p == NPOS - 1 and qslot == len(qbs) - 1)
                            for qt in qts:
                                if last:
                                    oproj_item(qt)()
                                else:
                                    put(("oproj", p, qt), oproj_item(qt))

                pend_av = []
                for b in batches:
                    need(kq_keys(p, b[4], b[5], b[2], b[3]))
                    pt = emit_scores(b)
                    drain_one()
                    if len(pend_av) >= PIPE_DEPTH:
                        b2, pt2 = pend_av.pop(0)
                        need(v_keys(p, b2[4], b2[5]))
                        emit_av(b2, pt2)
                    pend_av.append((b, pt))
                for b2, pt2 in pend_av:
                    need(v_keys(p, b2[4], b2[5]))
                    emit_av(b2, pt2)

                q0 += nq
            drain_all()
    nc.compile()  # bacc lowering (strips tile pseudo-insts for walrus)
    return nc


# --------------------------------------------------------------------------
# entry point
# --------------------------------------------------------------------------

_GRAPH_CACHE = {}


def prep_inputs(x, Wq, Wk, Wv, Wo, bo, offsets):
    x = np.ascontiguousarray(np.asarray(x, np.float32))
    sched = build_schedule(np.asarray(offsets))
    xkvT, validity, Wqp, Wkp, Wvp, Wop, scatter = shard_inputs(
        x, np.asarray(Wq, np.float32), np.asarray(Wk, np.float32),
        np.asarray(Wv, np.float32), np.asarray(Wo, np.float32), sched)
    in_maps = [
        dict(xkvT=xkvT[c], validity=validity[c],
             Wq=Wqp[c], Wk=Wkp[c], Wv=Wvp[c], Wo=Wop[c])
        for c in range(N_CORES)
    ]
    return sched, in_maps, scatter


def kernel(x, Wq, Wk, Wv, Wo, bo, offsets):
    from concourse.bass_utils import run_bass_kernel_spmd

    sched, in_maps, scatter = prep_inputs(x, Wq, Wk, Wv, Wo, bo, offsets)
    key = (tuple(sched["template"]), sched["NKV"], sched["NQP"])
    if key not in _GRAPH_CACHE:
        _GRAPH_CACHE[key] = build_graph(*key)
    nc = _GRAPH_CACHE[key]

    import time as _time
    _t0 = _time.monotonic()
    res = run_bass_kernel_spmd(nc, in_maps, core_ids=list(range(N_CORES)),
                               trace=bool(os.environ.get("KERNEL_TRACE")))
    kernel.last_run_s = _time.monotonic() - _t0
    kernel.last_results = res

    T = np.asarray(x).shape[0]
    out = np.zeros((T, D), np.float32)
    for (c, prow0, grow0, nrows) in scatter:
        out[grow0:grow0 + nrows] += np.asarray(
            res.results[c]["out"][prow0:prow0 + nrows], np.float32)
    out += np.asarray(bo, np.float32)
    return out

